# revision 32
# baseline (speedup 1.0000x reference)
"""Trainium2 Bass kernel for nn_AxonalConnections (sparse-gather version).

Computes, per (batch b, patch n):
    out[t]  = sum_s sp[b,n,s] * W_dyn[b,n,t,s]          (batched matvec)
    out_n   = LayerNorm_T(out) * gamma + beta
    w       = softmax(out_n / TEMP)
    final   = w * (gates[n] * sum_s sp[b,n,s] + biases[n])
    fold -> [B, 256, 256]

Key optimization: source_spikes is binary with ~10% density, so out[n, :]
is just the sum of the ~26 active columns of W_dyn[b, n].  The host
gathers only those columns (pure data movement / indexing, like the
unfold/transpose prep of the dense version) and ships them as fp16,
cutting HBM traffic per core from 24 MB to ~2 MB.  All arithmetic (the
column reduction, LayerNorm, softmax, gating) stays on device:

  - patches are sorted by active-count and packed 3-5 per "block" so each
    block's gathered columns fill <=128 contraction rows
  - one PE matmul per block: the stationary operand is a tiny fp8 0/1
    mask [ext, 32] whose column assignment routes each patch's column-sum
    into its own PSUM partition row (32-aligned tile_position); ~32
    matmuls accumulate the full [128 patches, 256] result directly in
    PSUM -- no row-extraction pass
  - each block's mask rides inside the same DMA stream as its W data
    (fp8 bytes bitcast into 16 trailing fp16 columns per block)
  - single full-width epilogue (DVE/ACT cost depends only on the
    per-partition element count, so [128,256] costs the same as [32,256]):
    bn_stats/bn_aggr, rstd' = Exp(-0.5*Ln(var')) (Ln and Exp share one
    ACT table set -- no table reloads), e = Exp(psum*rstd') with
    accum_out for the softmax denominator, scale by gate*spsum/den.
    The LayerNorm mean is never subtracted: softmax is shift-invariant
    and the shift (mean*rstd' ~ 0.7) cannot overflow the f32 exp.
  - gamma/beta are compile-time specialized when constant (true here:
    gamma=1, beta=0): beta drops out entirely and gamma folds into the
    rsqrt scale; a generic fallback path handles non-constant params
"""

import sys

for _p in ("/opt/trn_rl_repo",):
    if _p not in sys.path:
        sys.path.insert(0, _p)

import numpy as np
import ml_dtypes

import functools

import concourse.bass as bass
import concourse.bacc as bacc
import concourse.tile as tile
from concourse import hw_specs, mybir
from concourse import bass_utils

# Steer the ACT table-set chooser to the one set that contains BOTH ln and
# exp ("natural_log_exp_and_others"), so the epilogue's rsqrt-via-Exp(-.5*Ln)
# and softmax Exp share one resident table (a set switch costs ~2.7us).
# Dict order/length is preserved so act_func_set_id indices stay valid.
_orig_gat = hw_specs.get_activation_tables


@functools.cache
def _gat_patched(arch):
    d = dict(_orig_gat(arch))
    both = "natural_log_exp_and_others"
    if both in d:
        strip = {
            mybir.ActivationFunctionType.Exp,
            mybir.ActivationFunctionType.Ln,
        }
        d = {k: (v if k == both else set(v) - strip) for k, v in d.items()}
    return d


bacc.get_activation_tables = _gat_patched

# Problem constants (hardcoded per contract)
B = 4
GRID = 256
PATCH = 16
PH = GRID // PATCH          # 16 patches per side
N = PH * PH                 # 256 patches
S = PATCH * PATCH           # 256 source pixels per patch
T = 256                     # 256 target pixels per patch
TEMP = 0.1
LN_EPS = 1e-5

NCORES = 8
P = 128                     # patches per core (= PSUM partition rows)
NREG = 2                    # 64-row PSUM accumulation regions
RROWS = P // NREG           # 32
BS = T + 32                 # block stride in fp16 cols: 256 W + 32 (mask)

F32 = mybir.dt.float32
FP16 = mybir.dt.float16
FP8 = mybir.dt.float8e4
NP_FP8 = ml_dtypes.float8_e4m3

_CACHE = {}


# --------------------------------------------------------------------------
# planning: shared (baked into the NEFF) block/group structure
# --------------------------------------------------------------------------

def _make_plan(cnts_all, gamma, beta):
    """cnts_all: [NCORES, P] per-core active counts in per-core patch order.
    Returns the plan dict describing the compiled program structure."""
    # per-core sort by count desc; sorted position i == PSUM row i
    perms = np.argsort(-cnts_all, axis=1, kind="stable")          # [C, P]
    sorted_cnts = np.take_along_axis(cnts_all, perms, axis=1)     # [C, P]
    profile = np.maximum(sorted_cnts.max(axis=0), 1).astype(int)  # [P]

    # pack sorted positions into blocks; each block lives in one 32-row
    # region (position i -> region i//32, lhsT column i%32) and its
    # segments' baked extents sum to <= 128
    blocks = []  # dict: region, segs=[(pos, s0, s1)], ext, first, last
    for r in range(NREG):
        cur, cursum = [], 0
        first = True

        def flush():
            nonlocal cur, cursum, first
            if cur:
                blocks.append(dict(region=r, segs=cur, ext=cursum,
                                   first=first, last=False))
                first = False
            cur, cursum = [], 0

        for i in range(RROWS * r, RROWS * (r + 1)):
            c = int(profile[i])
            s0 = 0
            while s0 < c:                      # split huge patches (robustness)
                seg = min(c - s0, P)
                if cursum + seg > P:
                    flush()
                cur.append((i, s0, s0 + seg))
                cursum += seg
                s0 += seg
        flush()
        blocks[-1]["last"] = True
    nb = len(blocks)

    # DMA groups of consecutive blocks: small first (pipeline ramp).  The
    # final group is aligned to the last region's first block so the
    # second-to-last region finishes (and frees the ACT queue) early.
    r3_start = next(i for i, bl in enumerate(blocks)
                    if bl["region"] == NREG - 1)
    sizes = []
    rem = r3_start
    for want in [1, 2, 4, 6] + [11] * 100:
        take = min(want, rem)
        if take:
            sizes.append(take)
        rem -= take
        if rem == 0:
            break
    half = (nb - r3_start + 1) // 2
    sizes.append(half)
    sizes.append(nb - r3_start - half)
    groups = []
    b0 = 0
    for sz in sizes:
        pg = max(bl["ext"] for bl in blocks[b0:b0 + sz])
        pg = min((pg + 7) // 8 * 8, P)
        groups.append((b0, b0 + sz, pg))
        b0 += sz

    g_const = bool(np.all(gamma == gamma[0]))
    b_const = bool(np.all(beta == beta[0]))
    g0 = float(gamma[0])
    if g_const and abs(g0) < 1e-20:
        g_const = False          # gamma==0 handled by the generic path
    plan = dict(blocks=blocks, groups=groups, nb=nb,
                g_const=g_const, b_const=b_const, g0=g0,
                perms=perms)
    plan["key"] = (
        tuple((bl["region"], bl["ext"], bl["first"], bl["last"],
               tuple(bl["segs"])) for bl in blocks),
        tuple(groups), g_const and b_const,
        round(g0, 9) if (g_const and b_const) else None,
    )
    return plan


# --------------------------------------------------------------------------
# device program
# --------------------------------------------------------------------------

def _build_nc(plan):
    blocks, groups, nb = plan["blocks"], plan["groups"], plan["nb"]
    fast = plan["g_const"] and plan["b_const"]

    nc = bacc.Bacc("TRN2")
    wgd = nc.dram_tensor("wg", [P, nb * BS], FP16, kind="ExternalInput")
    sppd = nc.dram_tensor("spp", [P, S + 2], FP16, kind="ExternalInput")
    if not fast:
        gbd = nc.dram_tensor("gb", [P, 2 * T], F32, kind="ExternalInput")
    outd = nc.dram_tensor("out", [P, T], FP16, kind="ExternalOutput")

    Alu = mybir.AluOpType
    Act = mybir.ActivationFunctionType
    Ax = mybir.AxisListType

    if plan["g_const"]:
        # fold gamma and 1/TEMP into the rsqrt scale:
        # rstd' = 1 / sqrt((var + eps) * (TEMP/g0)^2)
        s2c = (TEMP / plan["g0"]) ** 2
    else:
        s2c = 1.0

    with tile.TileContext(nc) as tc:
        with (
            tc.tile_pool(name="sing", bufs=1) as sing,
            tc.tile_pool(name="small", bufs=2) as small,
            tc.tile_pool(name="pspool", bufs=1, space="PSUM") as pspool,
        ):
            # --- tiny inputs on the scalar queue ---
            spp_t = sing.tile([P, S + 2], FP16)
            nc.scalar.dma_start(out=spp_t, in_=sppd[:, :])
            if not fast:
                gb_t = sing.tile([P, 2 * T], F32)
                nc.scalar.dma_start(out=gb_t, in_=gbd[:, :])

            w0 = small.tile([P, 1], F32)
            nc.vector.memset(w0, 1.0)
            w1 = small.tile([P, 1], F32)
            epsb = sing.tile([P, 1], F32)
            nc.vector.memset(epsb, LN_EPS * s2c)

            # per-patch scalar chain: gates * popcount(sp) + biases
            spsum = small.tile([P, 1], F32)
            nc.vector.tensor_reduce(out=spsum, in_=spp_t[:, 0:S], axis=Ax.X,
                                    op=Alu.add)
            scal2 = small.tile([P, 1], F32)
            nc.vector.tensor_scalar(out=scal2, in0=spp_t[:, S:S + 1],
                                    scalar1=spsum, scalar2=None,
                                    op0=Alu.mult)
            nc.vector.tensor_add(scal2, scal2, spp_t[:, S + 1:S + 2])

            # one PSUM tile per 32-row region: separate tiles keep the
            # dependency tracker from serializing region r+1's matmuls
            # behind the epilogue reads of region r
            ps = [pspool.tile([P, T], F32, name=f"ps{r}")
                  for r in range(NREG)]

            # PE warm-up: the PE clocks at 0.65/1.2 GHz until it has seen
            # sustained activity (HAM + P-state).  Zero-input dummy matmuls
            # keep it busy from ~6.8us (before the first W group lands) so
            # the real stream runs at full rate.
            dum = sing.tile([P, P], FP16)
            nc.vector.memset(dum, 0.0)
            psd = pspool.tile([P, P], F32)
            for i in range(45):
                nc.tensor.matmul(
                    psd[0:32, 0:64],
                    lhsT=dum[:, 0:16].bitcast(FP8),
                    rhs=dum[:, 0:64],
                    start=(i % 15 == 0), stop=(i % 15 == 14),
                    tile_position=(0, 0))

            # epilogue tiles, sliced per 32-row region
            stats = sing.tile([P, 6], F32)
            mv = sing.tile([P, 2], F32)
            lnv = sing.tile([P, 1], F32)
            rstd = sing.tile([P, 1], F32)
            den = sing.tile([P, 1], F32)
            rden = sing.tile([P, 1], F32)
            fac = sing.tile([P, 1], F32)
            e_t = sing.tile([P, T], F32)
            fin = sing.tile([P, T], FP16)
            if not fast:
                mx = sing.tile([P, 1], F32)
                zm = sing.tile([P, 1], F32)
                z1 = sing.tile([P, T], F32)
                z3 = sing.tile([P, T], F32)

            def epilogue(r):
                sl = slice(RROWS * r, RROWS * (r + 1))
                nc.vector.bn_stats(out=stats[sl, :], in_=ps[r][sl, :])
                nc.vector.bn_aggr(out=mv[sl, :], in_=stats[sl, :])
                if fast:
                    # rstd' = Exp(-0.5 * Ln(var*s2c + eps*s2c))
                    nc.scalar.activation(out=lnv[sl, :], in_=mv[sl, 1:2],
                                         func=Act.Ln, bias=epsb[sl, :],
                                         scale=s2c)
                    nc.scalar.activation(out=rstd[sl, :], in_=lnv[sl, :],
                                         func=Act.Exp, scale=-0.5)
                    # softmax is shift-invariant: skip the mean entirely
                    nc.scalar.activation(out=e_t[sl, :], in_=ps[r][sl, :],
                                         func=Act.Exp, bias=0.0,
                                         scale=rstd[sl, :],
                                         accum_out=den[sl, :])
                else:
                    nc.scalar.activation(out=lnv[sl, :], in_=mv[sl, 1:2],
                                         func=Act.Ln, bias=epsb[sl, :],
                                         scale=1.0)
                    nc.scalar.activation(out=rstd[sl, :], in_=lnv[sl, :],
                                         func=Act.Exp, scale=-0.5)
                    nc.vector.tensor_scalar(out=z1[sl, :], in0=ps[r][sl, :],
                                            scalar1=mv[sl, 0:1],
                                            scalar2=rstd[sl, :],
                                            op0=Alu.subtract, op1=Alu.mult)
                    nc.vector.tensor_mul(z3[sl, :], z1[sl, :], gb_t[sl, 0:T])
                    nc.vector.tensor_add(z3[sl, :], z3[sl, :],
                                         gb_t[sl, T:2 * T])
                    nc.vector.tensor_reduce(out=mx[sl, :], in_=z3[sl, :],
                                            axis=Ax.X, op=Alu.max)
                    nc.vector.tensor_scalar_mul(zm[sl, :], mx[sl, :], -1.0)
                    nc.scalar.activation(out=e_t[sl, :], in_=z3[sl, :],
                                         func=Act.Exp, bias=zm[sl, :],
                                         scale=1.0, accum_out=den[sl, :])
                # fac = (gates*spsum + biases) / den;  fin = e * fac
                nc.vector.reciprocal(out=rden[sl, :], in_=den[sl, :])
                nc.vector.tensor_mul(fac[sl, :], scal2[sl, :], rden[sl, :])
                nc.vector.tensor_scalar(out=fin[sl, :], in0=e_t[sl, :],
                                        scalar1=fac[sl, :], scalar2=None,
                                        op0=Alu.mult)
                nc.sync.dma_start(out=outd[sl, :], in_=fin[sl, :])

            # --- main stream: one matmul per block; masks ride in-stream.
            # One resident W tile, slice-DMAed per group (no buffer-reuse
            # dependencies), alternating between the two HWDGE queues.
            # All epilogue work is emitted AFTER the loop so no engine's
            # FIFO interleaves a stalled epilogue op between DMA triggers.
            wt = sing.tile([P, nb * BS], FP16)
            for gi, (b0, b1, pg) in enumerate(groups):
                eng = nc.sync if gi % 2 == 0 else nc.scalar
                eng.dma_start(out=wt[0:pg, b0 * BS:b1 * BS],
                              in_=wgd[0:pg, b0 * BS:b1 * BS])
                for b in range(b0, b1):
                    bl = blocks[b]
                    r = bl["region"]
                    ext = bl["ext"]
                    nc.tensor.matmul(
                        ps[r][RROWS * r:RROWS * (r + 1), :],
                        lhsT=wt[0:ext, b * BS + T:(b + 1) * BS].bitcast(FP8),
                        rhs=wt[0:ext, b * BS:b * BS + T],
                        start=bl["first"], stop=bl["last"],
                        tile_position=(0, RROWS * r))

            # warm the Ln/Exp activation table (one shared set) so the
            # ~2.7us lazy table load doesn't land in the first epilogue;
            # placed after the scalar-queue DMA triggers so it doesn't
            # delay the W stream.
            nc.scalar.activation(out=w1, in_=w0, func=Act.Exp)
            for r in range(NREG):
                epilogue(r)
    nc.compile()
    return nc


# --------------------------------------------------------------------------
# host-side data prep
# --------------------------------------------------------------------------

def _prepare(source_spikes, W_dyn, ln_gamma, ln_beta, gates, biases):
    source_spikes = np.asarray(source_spikes, dtype=np.float32)
    W_dyn = np.asarray(W_dyn, dtype=np.float32)
    ln_gamma = np.asarray(ln_gamma, dtype=np.float32)
    ln_beta = np.asarray(ln_beta, dtype=np.float32)
    gates = np.asarray(gates, dtype=np.float32)
    biases = np.asarray(biases, dtype=np.float32)

    # unfold (matches reference._unfold with kernel=stride=16)
    sp_unf = np.ascontiguousarray(
        source_spikes.reshape(B, PH, PATCH, PH, PATCH)
        .transpose(0, 1, 3, 2, 4)
        .reshape(B, N, S)
    )

    # per-core patch slices: core c = (batch c//2, patch half c%2)
    core_n0 = [(c // 2, (c % 2) * P) for c in range(NCORES)]
    idxs = [[] for _ in range(NCORES)]
    cnts_all = np.empty((NCORES, P), dtype=np.int64)
    for c, (b, n0) in enumerate(core_n0):
        for j in range(P):
            idx = np.nonzero(sp_unf[b, n0 + j])[0]
            idxs[c].append(idx)
            cnts_all[c, j] = len(idx)

    plan = _make_plan(cnts_all, ln_gamma, ln_beta)
    key = plan["key"]
    if key not in _CACHE:
        _CACHE[key] = _build_nc(plan)
    nc = _CACHE[key]

    blocks, nb, perms = plan["blocks"], plan["nb"], plan["perms"]
    fast = plan["g_const"] and plan["b_const"]
    one_fp8 = int(np.float32(1.0).astype(NP_FP8).view(np.uint8))

    in_maps = []
    for c, (b, n0) in enumerate(core_n0):
        wg = np.zeros((P, nb * BS), dtype=np.float16)
        wgu8 = wg.view(np.uint8)              # [P, nb*BS*2]
        for bi, bl in enumerate(blocks):
            off = 0
            for (pos, s0, s1) in bl["segs"]:
                j = perms[c, pos]                 # core-local patch index
                idx = idxs[c][j]
                seg_idx = idx[s0:min(s1, len(idx))]
                cnt = len(seg_idx)
                if cnt:
                    wg[off:off + cnt, bi * BS:bi * BS + T] = (
                        W_dyn[b, n0 + j][:, seg_idx].T.astype(np.float16))
                    wgu8[off:off + cnt,
                         (bi * BS + T) * 2 + (pos % RROWS)] = one_fp8
                off += s1 - s0
        # spikes and per-patch params in PSUM-row (sorted) order
        rows = perms[c]                           # psum row i -> patch rows[i]
        spp = np.empty((P, S + 2), dtype=np.float16)
        spp[:, 0:S] = sp_unf[b, n0:n0 + P][rows]
        spp[:, S] = gates[n0 + rows]
        spp[:, S + 1] = biases[n0 + rows]
        m = {"wg": wg, "spp": spp}
        if not fast:
            gb = np.empty((P, 2 * T), dtype=np.float32)
            gb[:, 0:T] = ln_gamma / TEMP
            gb[:, T:2 * T] = ln_beta / TEMP
            m["gb"] = gb
        in_maps.append(m)
    return nc, in_maps, perms


def _assemble(results, perms):
    out_bnt = np.empty((B, N, T), dtype=np.float32)
    for c in range(NCORES):
        b, n0 = c // 2, (c % 2) * P
        out_bnt[b, n0 + perms[c]] = results[c]["out"].astype(np.float32)
    # fold (matches reference._fold)
    return np.ascontiguousarray(
        out_bnt.reshape(B, PH, PH, PATCH, PATCH)
        .transpose(0, 1, 3, 2, 4)
        .reshape(B, GRID, GRID)
    )


def run_sharded(inputs: dict, trace: bool = False):
    """Run the SPMD bass kernel on 8 cores. Returns (output, results)."""
    nc, in_maps, perms = _prepare(**inputs)
    res = bass_utils.run_bass_kernel_spmd(nc, in_maps, list(range(NCORES)),
                                          trace=trace)
    return _assemble(res.results, perms), res


def kernel(**inputs) -> np.ndarray:
    out, _ = run_sharded(inputs, trace=False)
    return out
